# revision 25
# baseline (speedup 1.0000x reference)
"""Trainium2 Bass kernel for nn_Capsule (dynamic routing capsule layer).

Math: with cij initialized to zeros, routing iteration 1 collapses to
cij = 1/32 (softmax of zeros), so the whole forward reduces to:
  T[b,j,d]   = sum_n u_hat[b,j,n,d]            (= rowsum(u[b]) @ W)
  S1         = sum(u_hat) = sum(T)
  S2         = sum(u_hat^2) = <W W^T, u^T u>   (feature Gram)
  s          = S1 * rsqrt(max(S2, 1e-12))      (global l2_normalize scalar)
  sjh2       = (s/32) * T ; sj2 = sjh2 * rsqrt(max(sum(sjh2^2), 1e-12))
  logits     = s * (u @ A[b]),  A[b][din,j] = sum_dd W[din,(j,dd)] sj2[b,j,dd]
  cij        = softmax_j(logits)
  G[b][j,:]  = sum_n cij[b,j,n] u[b,n,:]
  out        = squash(s * (G[b] fold W))
u_hat (256 MiB) is never materialized.  Sharding: data-parallel over
batch B (4 per core).  Cross-core reduction (Gram + rowsums -> 3
scalars) and the tiny O(B*J*D*DIN) fold/squash run on the host between
the two launches (in-kernel collectives cost ~63us first-use here, far
above the two-launch overhead).

Phase 1 reads the padded u1 layout (row + one-hot batch indicator, so a
single accumulating matmul chain yields Gram cols 0:128 and per-batch
rowsums cols 128:132).  Phase 2 reads only the NATURAL bf16 layout u2
(4.2 MiB instead of the old 8.5 MiB dual layout); the transposed copy
needed by the logits matmul is produced on-chip with the XBAR DMA
transpose (SBUF->SBUF, no HBM traffic).  Matmul operands are bf16
(fp32 accumulation in PSUM, rel err ~4e-3).
"""

import numpy as np

import concourse.bacc as bacc
import concourse.mybir as mybir
import concourse.tile as tile
from concourse.bass import ts
from concourse.bass_utils import run_bass_kernel_spmd

N_CORES = 8
B, N, DIN = 32, 4096, 128
J, D = 32, 16
K = J * D  # 512
B_LOC = B // N_CORES          # 4 batches per core
CPB = N // 128                # 32 chunks of 128 rows per batch
E1 = DIN + B_LOC              # 132: row + one-hot batch indicator
NH = 2 * B_LOC                # 8 half-batch groups
CPH = CPB // 2                # 16 chunks per half-batch
F32 = mybir.dt.float32
BF16 = mybir.dt.bfloat16
FP8 = mybir.dt.float8e4
AX = mybir.AxisListType
ALU = mybir.AluOpType
ACTF = mybir.ActivationFunctionType

NWARM1 = 28                   # phase-1 PE warmup matmuls
NWARM2 = 28                   # phase-2 PE warmup matmuls
CPP = 8                       # chunks per piece (phase-2 softmax granularity)
NP = (B_LOC * CPB) // CPP     # 16 pieces
PPB = CPB // CPP              # 4 pieces per batch
LAG = 4                       # pieces of logits emitted ahead of their chain

PROFILE = False
LAST_TIMES = {}

_CACHE = {}


def _new_bass():
    return bacc.Bacc(
        "TRN2",
        target_bir_lowering=False,
        debug=False,
        enable_asserts=False,
        num_devices=N_CORES,
    )


def _emit_warmup(nc, sbpool, pspool, n):
    """Dummy back-to-back matmuls during the initial DMA wait: the PE
    HAM clock-gate needs ~3.4us of sustained activity to unthrottle
    from 1.2 to 2.4 GHz, so burn the otherwise-idle preamble window on
    garbage matmuls and run the real ones warm."""
    wsb = sbpool.tile([128, 128], BF16, tag="wsb", name="wsb")
    nc.vector.memset(wsb[:], 1.0)
    wps = pspool.tile([128, 128], F32, tag="wps", name="wps")
    for i in range(n):
        nc.tensor.matmul(wps[:], wsb[:], wsb[:], start=True, stop=True)


def _build_phase1():
    """Per core: one accumulating matmul chain over 128 row-chunks of
    the padded u layout -> [C | R] = [128, 132] (Gram + per-batch
    rowsums)."""
    nc = _new_bass()
    u_d = nc.dram_tensor("u1", [128, B_LOC * CPB * E1], BF16, kind="ExternalInput")
    o_d = nc.dram_tensor("p1", [128, E1], F32, kind="ExternalOutput")

    with tile.TileContext(nc) as tc:
        with (
            tc.tile_pool(name="upool", bufs=1) as upool,
            tc.tile_pool(name="psp", bufs=1, space="PSUM") as psp,
            tc.tile_pool(name="sbp", bufs=1) as sbp,
            tc.tile_pool(name="wup", bufs=1, space="PSUM") as wup,
        ):
            _emit_warmup(nc, sbp, wup, NWARM1)
            # 8 half-batch DMAs on the two HWDGE rings; each partition's
            # DRAM source is one contiguous 16*132*2 B run.
            ugs = []
            for h in range(NH):
                ug = upool.tile([128, CPH * E1], BF16, tag=f"ug{h}", name=f"ug{h}")
                ugs.append(ug)
                eng = nc.sync if h % 2 == 0 else nc.scalar
                eng.dma_start(ug[:], u_d.ap()[:, ts(h, CPH * E1)])

            acc = psp.tile([128, E1], F32, tag="acc", name="acc")
            for c in range(B_LOC * CPB):
                h, cl = divmod(c, CPH)
                view = ugs[h][:].rearrange("p (c e) -> p c e", e=E1)[:, cl, :]
                nc.tensor.matmul(
                    acc[:],
                    view[:, 0:DIN],
                    view,
                    start=(c == 0),
                    stop=(c == B_LOC * CPB - 1),
                )

            outsb = sbp.tile([128, E1], F32, tag="outsb", name="outsb")
            nc.scalar.copy(outsb[:], acc[:])
            nc.sync.dma_start(o_d.ap(), outsb[:])

    nc.compile()
    return nc


def _build_phase2():
    """Per core: logits -> softmax -> G.

    Inputs: u2, the natural bf16 layout [128, 4*32*128] (partition p,
    batch b, chunk c at cols (b*32+c)*128, holding u[4i+b, 32p+c, :]);
    ut, the host-transposed fp8e4 copy (ut[d, (b*32+c)*128+m] =
    u[4i+b, 32m+c, d]) used only as the logits stationary operand, where
    fp8's ~2% element error only perturbs softmax weights by ~0.5%.
    DMA is ring-balanced: sync carries all of ut (2.1 MiB) + the last
    two u2 groups; scalar carries the first six u2 groups (3.15 MiB
    per ring).  Work is chained in pieces of CPP=8 chunks: logits
    (stationary = ut chunk fp8, moving = A[b] 32 cols bf16), exp on
    ACT, softmax reduce/mult on DVE, accumulating G matmuls per batch.
    """
    nc = _new_bass()
    u_d = nc.dram_tensor("u2", [128, B_LOC * CPB * DIN], BF16, kind="ExternalInput")
    t_d = nc.dram_tensor("ut", [128, B_LOC * CPB * DIN], FP8, kind="ExternalInput")
    a_d = nc.dram_tensor("A", [DIN, B_LOC * J], BF16, kind="ExternalInput")  # s*A
    # out row 32*b+j holds G[b, j, :] (length-128 din)
    o_d = nc.dram_tensor("out", [128, DIN], F32, kind="ExternalOutput")

    with tile.TileContext(nc) as tc:
        with (
            tc.tile_pool(name="const", bufs=1) as cstp,
            tc.tile_pool(name="upool", bufs=1) as upool,
            tc.tile_pool(name="utp", bufs=1) as utp,
            tc.tile_pool(name="expp", bufs=8) as expp,
            tc.tile_pool(name="zgp", bufs=8) as zgp,
            tc.tile_pool(name="zrp", bufs=8) as zrp,
            tc.tile_pool(name="cijp", bufs=8) as cijp,
            tc.tile_pool(name="sbt", bufs=1) as sbt,
            tc.tile_pool(name="plp", bufs=6, space="PSUM") as plp,
            tc.tile_pool(name="tlp", bufs=1, space="PSUM") as tlp,
            tc.tile_pool(name="wup", bufs=1, space="PSUM") as wup,
        ):
            # small load first so it doesn't queue behind the u loads
            a_sb = cstp.tile([128, B_LOC * J], BF16, tag="a_sb", name="a_sb")
            nc.scalar.dma_start(a_sb[:], a_d.ap())
            _emit_warmup(nc, cstp, wup, NWARM2)

            # DMA issue plan.  The 16 DMA engines drain striped descriptors
            # in enqueue order, so bytes must be ISSUED globally in need
            # order: utg0..3 (gate the first logits) first on sync, then
            # the u2 groups.  ACT carries ~1.5 MiB for ring balance, but
            # its issues are interleaved between the early exps (emitted
            # inside the piece loop below) so its late-needed bytes don't
            # jump the queue; sync (no compute) may stall on ring-full
            # freely.
            # Each HWDGE ring feeds only half the DMA queues (~215 GB/s),
            # so full HBM rate needs both rings busy from t=0 AND bytes
            # flowing in need order on each.  ut (gates all logits) is
            # split across both rings up front; u2 groups follow, with
            # ACT's share issued from slots between exps (4 early issues
            # never ring-full-stall the exp chain).  ~3.15 MiB per ring.
            utgs = [None] * NH
            ugs = [None] * NH
            for h in range(NH):
                ugs[h] = upool.tile(
                    [128, CPH * DIN], BF16, tag=f"ug{h}", name=f"ug{h}"
                )
            utgB45 = utp.tile([128, 2 * CPH * DIN], FP8, tag="utgB45", name="utgB45")
            utgB67 = utp.tile([128, 2 * CPH * DIN], FP8, tag="utgB67", name="utgB67")
            for h in (4, 5):
                utgs[h] = (utgB45, (h - 4) * CPH * DIN)
            for h in (6, 7):
                utgs[h] = (utgB67, (h - 6) * CPH * DIN)
            # ACT ring: utg1, utg3, utgB67 (1.05 MiB, issued immediately)
            for h in (1, 3):
                utg = utp.tile([128, CPH * DIN], FP8, tag=f"utg{h}", name=f"utg{h}")
                utgs[h] = (utg, 0)
                nc.scalar.dma_start(utg[:], t_d.ap()[:, ts(h, CPH * DIN)])
            nc.scalar.dma_start(utgB67[:], t_d.ap()[:, 6 * CPH * DIN :])
            # sync ring: utg0, utg2, utgB45, ug0..ug3
            for h in (0, 2):
                utg = utp.tile([128, CPH * DIN], FP8, tag=f"utg{h}", name=f"utg{h}")
                utgs[h] = (utg, 0)
                nc.sync.dma_start(utg[:], t_d.ap()[:, ts(h, CPH * DIN)])
            nc.sync.dma_start(
                utgB45[:], t_d.ap()[:, 4 * CPH * DIN : 6 * CPH * DIN]
            )
            for h in range(4):
                nc.sync.dma_start(ugs[h][:], u_d.ap()[:, ts(h, CPH * DIN)])

            def emit_act_dma(p):
                if p == 2:
                    nc.scalar.dma_start(ugs[4][:], u_d.ap()[:, ts(4, CPH * DIN)])
                elif p == 5:
                    nc.scalar.dma_start(ugs[5][:], u_d.ap()[:, ts(5, CPH * DIN)])
                elif p == 8:
                    nc.scalar.dma_start(ugs[6][:], u_d.ap()[:, ts(6, CPH * DIN)])
                elif p == 11:
                    nc.scalar.dma_start(ugs[7][:], u_d.ap()[:, ts(7, CPH * DIN)])

            psg = tlp.tile([128, DIN], F32, tag="psg", name="psg")  # G accumulator

            pls = [None] * NP

            def emit_logits(p):
                b = p // PPB
                h, half = divmod(p, 2)
                utg, uoff = utgs[h]
                # full PSUM bank per piece: a shared bank between PE logits
                # writes and ACT exp reads serializes the pipeline
                plf = plp.tile([128, 512], F32, tag="pl", name=f"pl{p}")
                pls[p] = plf
                for cl in range(CPP):
                    o = uoff + half * CPP * 128 + cl * 128
                    nc.tensor.matmul(
                        pls[p][:, ts(cl, J)],
                        utg[:, o : o + 128],
                        a_sb[:, ts(b, J)],
                        start=True,
                        stop=True,
                    )

            def emit_chain(p):
                # softmax over j (free axis) + G matmuls for piece p
                b, pl_in_b = divmod(p, PPB)
                h, half = divmod(p, 2)
                eg = expp.tile([128, CPP * J], F32, tag="eg", name=f"eg{p}")
                nc.scalar.activation(eg[:], pls[p][:, 0 : CPP * J], ACTF.Exp)
                zg = zgp.tile([128, CPP], F32, tag="zg", name=f"zg{p}")
                nc.vector.reduce_sum(
                    zg[:], eg[:].rearrange("q (c j) -> q c j", j=J), axis=AX.X
                )
                zr = zrp.tile([128, CPP], F32, tag="zr", name=f"zr{p}")
                nc.vector.reciprocal(zr[:], zg[:])
                cg = cijp.tile([128, CPP * J], BF16, tag="cg", name=f"cg{p}")
                # multiply on the otherwise-idle Pool engine so DVE's
                # reduce+reciprocal keep pace with the DMA stream
                nc.gpsimd.tensor_tensor(
                    cg[:].rearrange("q (c j) -> q c j", j=J),
                    eg[:].rearrange("q (c j) -> q c j", j=J),
                    zr[:].unsqueeze(2).broadcast_to([128, CPP, J]),
                    op=ALU.mult,
                )
                for cl in range(CPP):
                    c_in_b = pl_in_b * CPP + cl
                    nc.tensor.matmul(
                        psg[ts(b, J), :],
                        cg[:, ts(cl, J)],
                        ugs[h][:, half * CPP * 128 + cl * 128 : half * CPP * 128 + (cl + 1) * 128],
                        start=(c_in_b == 0),
                        stop=(c_in_b == CPB - 1),
                        tile_position=(0, J * b),
                    )

            for p in range(NP):
                emit_logits(p)
                emit_act_dma(p)
                if p >= LAG:
                    emit_chain(p - LAG)
            for p in range(NP - LAG, NP):
                emit_chain(p)

            gout = sbt.tile([128, DIN], F32, tag="gout", name="gout")
            nc.scalar.copy(gout[:], psg[:])
            nc.sync.dma_start(o_d.ap(), gout[:])

    nc.compile()
    return nc


def _get(name):
    if name not in _CACHE:
        if name == "p1":
            _CACHE[name] = _build_phase1()
        else:
            _CACHE[name] = _build_phase2()
    return _CACHE[name]


def kernel(u, W):
    import ml_dtypes

    bf16 = ml_dtypes.bfloat16
    u = np.ascontiguousarray(u, dtype=np.float32)
    W = np.ascontiguousarray(W, dtype=np.float32)
    W0 = np.ascontiguousarray(W[0])  # [128, 512]
    ub = u.astype(bf16)

    # padded re-blocked layout: u1[i][p, ((b,c),e)] = [u[4i+b, 32p+c, :] | e_b]
    up = np.zeros((B, N, E1), dtype=bf16)
    up[:, :, :DIN] = ub
    for b in range(B_LOC):
        up[b::B_LOC, :, DIN + b] = 1.0  # batch index within the core shard
    up = up.reshape(N_CORES, B_LOC, 128, CPB, E1).transpose(0, 2, 1, 3, 4)
    u1 = [np.ascontiguousarray(up[i].reshape(128, B_LOC * CPB * E1))
          for i in range(N_CORES)]
    # natural layout for phase 2 (same row permutation, no padding):
    # u2[i][p, (b*32+c)*128 + e] = u[4i+b, 32p+c, e]
    u2v = ub.reshape(N_CORES, B_LOC, 128, CPB, DIN).transpose(0, 2, 1, 3, 4)
    u2 = [np.ascontiguousarray(u2v[i].reshape(128, B_LOC * CPB * DIN))
          for i in range(N_CORES)]
    # transposed fp8 copy with the same row permutation:
    # ut[i][d, (b*32+c)*128 + m] = u[4i+b, 32m+c, d]
    fp8 = ml_dtypes.float8_e4m3fn
    ut3 = ub.astype(fp8).reshape(N_CORES, B_LOC, 128, CPB, DIN).transpose(
        0, 4, 1, 3, 2
    )
    utl = [np.ascontiguousarray(ut3[i].reshape(128, B_LOC * CPB * DIN))
           for i in range(N_CORES)]

    # ---- phase 1: per-core Gram + rowsums ----
    nc1 = _get("p1")
    r1 = run_bass_kernel_spmd(
        nc1,
        [{"u1": u1[i]} for i in range(N_CORES)],
        core_ids=list(range(N_CORES)),
        trace=PROFILE,
    )
    if PROFILE:
        LAST_TIMES["phase1_ns"] = r1.exec_time_ns

    # ---- host: global scalar reduction (the "all-reduce" of 3 scalars) ----
    C = np.zeros((128, 128), dtype=np.float64)
    Rall = np.empty((128, B), dtype=np.float64)
    for i in range(N_CORES):
        p = r1.results[i]["p1"].astype(np.float64)
        C += p[:, :DIN]
        Rall[:, i * B_LOC : (i + 1) * B_LOC] = p[:, DIN:E1]
    W0d = W0.astype(np.float64)
    M = W0d @ W0d.T
    S2 = float(np.vdot(M, C))
    T = Rall.T @ W0d  # [B, 512]
    S1 = float(T.sum())
    s = S1 / np.sqrt(max(S2, 1e-12))
    sjh2 = (s / J) * T
    n2 = float((sjh2 * sjh2).sum())
    sj2 = (sjh2 / np.sqrt(max(n2, 1e-12))).reshape(B, J, D)
    # A[b][din, j] = sum_dd W0[din, j*16+dd] * sj2[b, j, dd];  fold s in
    A = np.einsum("dje,bje->bdj", W0d.reshape(DIN, J, D), sj2)
    As = (s * A).astype(bf16)  # [B, 128, 32]

    # ---- phase 2: logits/softmax/G ----
    nc2 = _get("p2")
    in2 = [
        {
            "u2": u2[i],
            "ut": utl[i],
            "A": np.ascontiguousarray(
                As[i * B_LOC : (i + 1) * B_LOC].transpose(1, 0, 2).reshape(DIN, -1)
            ),
        }
        for i in range(N_CORES)
    ]
    r2 = run_bass_kernel_spmd(
        nc2, in2, core_ids=list(range(N_CORES)), trace=PROFILE
    )
    if PROFILE:
        LAST_TIMES["phase2_ns"] = r2.exec_time_ns

    # ---- host: tiny fold + squash (O(B*J*D*DIN)) ----
    G = np.concatenate(
        [r2.results[i]["out"].astype(np.float64).reshape(B_LOC, J, DIN)
         for i in range(N_CORES)]
    )  # [B, J, 128]
    sjh3 = s * np.einsum("bjd,dje->bje", G, W0d.reshape(DIN, J, D))
    s2 = (sjh3 * sjh3).sum(axis=-1, keepdims=True) + 1e-7
    out = (np.sqrt(s2) / (1.0 + s2)) * sjh3
    return out.astype(np.float32)


# revision 28
# speedup vs baseline: 1.1000x; 1.1000x over previous
"""Trainium2 Bass kernel for nn_Capsule (dynamic routing capsule layer).

Math: with cij initialized to zeros, routing iteration 1 collapses to
cij = 1/32 (softmax of zeros), so the whole forward reduces to:
  T[b,j,d]   = sum_n u_hat[b,j,n,d]            (= rowsum(u[b]) @ W)
  S1         = sum(u_hat) = sum(T)
  S2         = sum(u_hat^2) = <W W^T, u^T u>   (feature Gram)
  s          = S1 * rsqrt(max(S2, 1e-12))      (global l2_normalize scalar)
  sjh2       = (s/32) * T ; sj2 = sjh2 * rsqrt(max(sum(sjh2^2), 1e-12))
  logits     = s * (u @ A[b]),  A[b][din,j] = sum_dd W[din,(j,dd)] sj2[b,j,dd]
  cij        = softmax_j(logits)
  G[b][j,:]  = sum_n cij[b,j,n] u[b,n,:]
  out        = squash(s * (G[b] fold W))
u_hat (256 MiB) is never materialized.  Sharding: data-parallel over
batch B (4 per core).  Cross-core reduction (Gram + rowsums -> 3
scalars) and the tiny O(B*J*D*DIN) fold/squash run on the host between
the two launches (in-kernel collectives cost ~63us first-use here, far
above the two-launch overhead).

Phase 1 reads the padded u1 layout (row + one-hot batch indicator, so a
single accumulating matmul chain yields Gram cols 0:128 and per-batch
rowsums cols 128:132).  Phase 2 reads only the NATURAL bf16 layout u2
(4.2 MiB instead of the old 8.5 MiB dual layout); the transposed copy
needed by the logits matmul is produced on-chip with the XBAR DMA
transpose (SBUF->SBUF, no HBM traffic).  Matmul operands are bf16
(fp32 accumulation in PSUM, rel err ~4e-3).
"""

import numpy as np

import concourse.bacc as bacc
import concourse.mybir as mybir
import concourse.tile as tile
from concourse.bass import ts
from concourse.bass_utils import run_bass_kernel_spmd

N_CORES = 8
B, N, DIN = 32, 4096, 128
J, D = 32, 16
K = J * D  # 512
B_LOC = B // N_CORES          # 4 batches per core
CPB = N // 128                # 32 chunks of 128 rows per batch
E1 = DIN + B_LOC              # 132: row + one-hot batch indicator
NH = 2 * B_LOC                # 8 half-batch groups
CPH = CPB // 2                # 16 chunks per half-batch
F32 = mybir.dt.float32
BF16 = mybir.dt.bfloat16
FP8 = mybir.dt.float8e4
AX = mybir.AxisListType
ALU = mybir.AluOpType
ACTF = mybir.ActivationFunctionType

NWARM1 = 28                   # phase-1 PE warmup matmuls
NWARM2 = 28                   # phase-2 PE warmup matmuls
CPP = 8                       # chunks per piece (phase-2 softmax granularity)
NP = (B_LOC * CPB) // CPP     # 16 pieces
PPB = CPB // CPP              # 4 pieces per batch
LAG = 2                       # pieces of logits emitted ahead of their chain

PROFILE = False
LAST_TIMES = {}

_CACHE = {}


def _new_bass():
    return bacc.Bacc(
        "TRN2",
        target_bir_lowering=False,
        debug=False,
        enable_asserts=False,
        num_devices=N_CORES,
    )


def _emit_warmup(nc, sbpool, pspool, n):
    """Dummy back-to-back matmuls during the initial DMA wait: the PE
    HAM clock-gate needs ~3.4us of sustained activity to unthrottle
    from 1.2 to 2.4 GHz, so burn the otherwise-idle preamble window on
    garbage matmuls and run the real ones warm."""
    wsb = sbpool.tile([128, 128], BF16, tag="wsb", name="wsb")
    nc.vector.memset(wsb[:], 1.0)
    wps = pspool.tile([128, 128], F32, tag="wps", name="wps")
    for i in range(n):
        nc.tensor.matmul(wps[:], wsb[:], wsb[:], start=True, stop=True)


def _build_phase1():
    """Per core: one accumulating fp8 matmul chain over 128 row-chunks
    of the natural layout -> C = [128, 128] (feature Gram).  fp8 is
    safe here: C only feeds the global scalar S2 = <W W^T, C>, a large
    positive sum where fp8's ~2% element errors average out to <0.1%.
    The per-batch rowsums (which feed the cancellation-heavy S1) are
    computed exactly on the host instead."""
    nc = _new_bass()
    u_d = nc.dram_tensor("u3", [128, B_LOC * CPB * DIN], FP8, kind="ExternalInput")
    o_d = nc.dram_tensor("p1", [128, DIN], F32, kind="ExternalOutput")

    with tile.TileContext(nc) as tc:
        with (
            tc.tile_pool(name="upool", bufs=1) as upool,
            tc.tile_pool(name="psp", bufs=1, space="PSUM") as psp,
            tc.tile_pool(name="sbp", bufs=1) as sbp,
            tc.tile_pool(name="wup", bufs=1, space="PSUM") as wup,
        ):
            _emit_warmup(nc, sbp, wup, NWARM1)
            # 8 half-batch DMAs on the two HWDGE rings; each partition's
            # DRAM source is one contiguous 16*128 B run.
            ugs = []
            for h in range(NH):
                ug = upool.tile([128, CPH * DIN], FP8, tag=f"ug{h}", name=f"ug{h}")
                ugs.append(ug)
                eng = nc.sync if h % 2 == 0 else nc.scalar
                eng.dma_start(ug[:], u_d.ap()[:, ts(h, CPH * DIN)])

            acc = psp.tile([128, DIN], F32, tag="acc", name="acc")
            for c in range(B_LOC * CPB):
                h, cl = divmod(c, CPH)
                view = ugs[h][:].rearrange("p (c e) -> p c e", e=DIN)[:, cl, :]
                nc.tensor.matmul(
                    acc[:],
                    view,
                    view,
                    start=(c == 0),
                    stop=(c == B_LOC * CPB - 1),
                )

            outsb = sbp.tile([128, DIN], F32, tag="outsb", name="outsb")
            nc.scalar.copy(outsb[:], acc[:])
            nc.sync.dma_start(o_d.ap(), outsb[:])

    nc.compile()
    return nc


def _build_phase2():
    """Per core: logits -> softmax -> G.

    Inputs: u2, the natural bf16 layout [128, 4*32*128] (partition p,
    batch b, chunk c at cols (b*32+c)*128, holding u[4i+b, 32p+c, :]);
    ut, the host-transposed fp8e4 copy (ut[d, (b*32+c)*128+m] =
    u[4i+b, 32m+c, d]) used only as the logits stationary operand, where
    fp8's ~2% element error only perturbs softmax weights by ~0.5%.
    DMA is ring-balanced: sync carries all of ut (2.1 MiB) + the last
    two u2 groups; scalar carries the first six u2 groups (3.15 MiB
    per ring).  Work is chained in pieces of CPP=8 chunks: logits
    (stationary = ut chunk fp8, moving = A[b] 32 cols bf16), exp on
    ACT, softmax reduce/mult on DVE, accumulating G matmuls per batch.
    """
    nc = _new_bass()
    u_d = nc.dram_tensor("u2", [128, B_LOC * CPB * DIN], BF16, kind="ExternalInput")
    t_d = nc.dram_tensor("ut", [128, B_LOC * CPB * DIN], FP8, kind="ExternalInput")
    a_d = nc.dram_tensor("A", [DIN, B_LOC * J], BF16, kind="ExternalInput")  # s*A
    # out row 32*b+j holds G[b, j, :] (length-128 din)
    o_d = nc.dram_tensor("out", [128, DIN], F32, kind="ExternalOutput")

    with tile.TileContext(nc) as tc:
        with (
            tc.tile_pool(name="const", bufs=1) as cstp,
            tc.tile_pool(name="upool", bufs=1) as upool,
            tc.tile_pool(name="utp", bufs=1) as utp,
            tc.tile_pool(name="expp", bufs=8) as expp,
            tc.tile_pool(name="zgp", bufs=8) as zgp,
            tc.tile_pool(name="zrp", bufs=8) as zrp,
            tc.tile_pool(name="cijp", bufs=8) as cijp,
            tc.tile_pool(name="sbt", bufs=1) as sbt,
            tc.tile_pool(name="plp", bufs=4, space="PSUM") as plp,
            tc.tile_pool(name="tlp", bufs=1, space="PSUM") as tlp,
            tc.tile_pool(name="wup", bufs=1, space="PSUM") as wup,
        ):
            # small load first so it doesn't queue behind the u loads
            a_sb = cstp.tile([128, B_LOC * J], BF16, tag="a_sb", name="a_sb")
            nc.scalar.dma_start(a_sb[:], a_d.ap())
            _emit_warmup(nc, cstp, wup, NWARM2)

            # DMA issue plan.  The 16 DMA engines drain striped descriptors
            # in enqueue order, so bytes must be ISSUED globally in need
            # order: utg0..3 (gate the first logits) first on sync, then
            # the u2 groups.  ACT carries ~1.5 MiB for ring balance, but
            # its issues are interleaved between the early exps (emitted
            # inside the piece loop below) so its late-needed bytes don't
            # jump the queue; sync (no compute) may stall on ring-full
            # freely.
            # v6 plan: ut front-loaded on sync (utg0..3 fine-grained,
            # then u2 groups 0..6); ACT carries ut's back half and ug7,
            # issued from slots between the early exps.
            utgs = [None] * NH
            ugs = [None] * NH
            for h in range(NH):
                ugs[h] = upool.tile(
                    [128, CPH * DIN], BF16, tag=f"ug{h}", name=f"ug{h}"
                )
            utgB45 = utp.tile([128, 2 * CPH * DIN], FP8, tag="utgB45", name="utgB45")
            utgB67 = utp.tile([128, 2 * CPH * DIN], FP8, tag="utgB67", name="utgB67")
            for h in (4, 5):
                utgs[h] = (utgB45, (h - 4) * CPH * DIN)
            for h in (6, 7):
                utgs[h] = (utgB67, (h - 6) * CPH * DIN)
            for h in range(4):
                utg = utp.tile([128, CPH * DIN], FP8, tag=f"utg{h}", name=f"utg{h}")
                utgs[h] = (utg, 0)
                nc.sync.dma_start(utg[:], t_d.ap()[:, ts(h, CPH * DIN)])
            for h in range(7):
                nc.sync.dma_start(ugs[h][:], u_d.ap()[:, ts(h, CPH * DIN)])

            def emit_act_dma(p):
                if p == 2:
                    nc.scalar.dma_start(
                        utgB45[:], t_d.ap()[:, 4 * CPH * DIN : 6 * CPH * DIN]
                    )
                elif p == 5:
                    nc.scalar.dma_start(utgB67[:], t_d.ap()[:, 6 * CPH * DIN :])
                elif p == 9:
                    nc.scalar.dma_start(ugs[7][:], u_d.ap()[:, ts(7, CPH * DIN)])

            psg = tlp.tile([128, DIN], F32, tag="psg", name="psg")  # G accumulator

            pls = [None] * NP

            def emit_logits(p):
                b = p // PPB
                h, half = divmod(p, 2)
                utg, uoff = utgs[h]
                # full PSUM bank per piece: a shared bank between PE logits
                # writes and ACT exp reads serializes the pipeline
                plf = plp.tile([128, 512], F32, tag="pl", name=f"pl{p}")
                pls[p] = plf
                for cl in range(CPP):
                    o = uoff + half * CPP * 128 + cl * 128
                    nc.tensor.matmul(
                        pls[p][:, ts(cl, J)],
                        utg[:, o : o + 128],
                        a_sb[:, ts(b, J)],
                        start=True,
                        stop=True,
                    )

            def emit_chain(p):
                # softmax over j (free axis) + G matmuls for piece p
                b, pl_in_b = divmod(p, PPB)
                h, half = divmod(p, 2)
                eg = expp.tile([128, CPP * J], F32, tag="eg", name=f"eg{p}")
                nc.scalar.activation(eg[:], pls[p][:, 0 : CPP * J], ACTF.Exp)
                zg = zgp.tile([128, CPP], F32, tag="zg", name=f"zg{p}")
                nc.vector.reduce_sum(
                    zg[:], eg[:].rearrange("q (c j) -> q c j", j=J), axis=AX.X
                )
                zr = zrp.tile([128, CPP], F32, tag="zr", name=f"zr{p}")
                nc.vector.reciprocal(zr[:], zg[:])
                cg = cijp.tile([128, CPP * J], BF16, tag="cg", name=f"cg{p}")
                # multiply on the otherwise-idle Pool engine so DVE's
                # reduce+reciprocal keep pace with the DMA stream
                nc.gpsimd.tensor_tensor(
                    cg[:].rearrange("q (c j) -> q c j", j=J),
                    eg[:].rearrange("q (c j) -> q c j", j=J),
                    zr[:].unsqueeze(2).broadcast_to([128, CPP, J]),
                    op=ALU.mult,
                )
                for cl in range(CPP):
                    c_in_b = pl_in_b * CPP + cl
                    nc.tensor.matmul(
                        psg[ts(b, J), :],
                        cg[:, ts(cl, J)],
                        ugs[h][:, half * CPP * 128 + cl * 128 : half * CPP * 128 + (cl + 1) * 128],
                        start=(c_in_b == 0),
                        stop=(c_in_b == CPB - 1),
                        tile_position=(0, J * b),
                    )

            for p in range(NP):
                emit_logits(p)
                emit_act_dma(p)
                if p >= LAG:
                    emit_chain(p - LAG)
            for p in range(NP - LAG, NP):
                emit_chain(p)

            gout = sbt.tile([128, DIN], F32, tag="gout", name="gout")
            nc.scalar.copy(gout[:], psg[:])
            nc.sync.dma_start(o_d.ap(), gout[:])

    nc.compile()
    return nc


def _get(name):
    if name not in _CACHE:
        if name == "p1":
            _CACHE[name] = _build_phase1()
        else:
            _CACHE[name] = _build_phase2()
    return _CACHE[name]


def kernel(u, W):
    import ml_dtypes

    bf16 = ml_dtypes.bfloat16
    u = np.ascontiguousarray(u, dtype=np.float32)
    W = np.ascontiguousarray(W, dtype=np.float32)
    W0 = np.ascontiguousarray(W[0])  # [128, 512]
    ub = u.astype(bf16)
    fp8 = ml_dtypes.float8_e4m3fn
    uf8 = ub.astype(fp8)

    # natural layout (row r of batch b lands at partition r%128... the
    # permutation u[4i+b, 32p+c, e] -> [i][p, (b*32+c)*128 + e] is shared
    # by all three device tensors, so logits/softmax/G stay consistent)
    u2v = ub.reshape(N_CORES, B_LOC, 128, CPB, DIN).transpose(0, 2, 1, 3, 4)
    u2 = [np.ascontiguousarray(u2v[i].reshape(128, B_LOC * CPB * DIN))
          for i in range(N_CORES)]
    # fp8 natural copy for the phase-1 Gram chain
    u3v = uf8.reshape(N_CORES, B_LOC, 128, CPB, DIN).transpose(0, 2, 1, 3, 4)
    u3 = [np.ascontiguousarray(u3v[i].reshape(128, B_LOC * CPB * DIN))
          for i in range(N_CORES)]
    # transposed fp8 copy: ut[i][d, (b*32+c)*128 + m] = u[4i+b, 32m+c, d]
    ut3 = uf8.reshape(N_CORES, B_LOC, 128, CPB, DIN).transpose(0, 4, 1, 3, 2)
    utl = [np.ascontiguousarray(ut3[i].reshape(128, B_LOC * CPB * DIN))
           for i in range(N_CORES)]

    # ---- phase 1: per-core Gram ----
    nc1 = _get("p1")
    r1 = run_bass_kernel_spmd(
        nc1,
        [{"u3": u3[i]} for i in range(N_CORES)],
        core_ids=list(range(N_CORES)),
        trace=PROFILE,
    )
    if PROFILE:
        LAST_TIMES["phase1_ns"] = r1.exec_time_ns

    # ---- host: global scalar reduction (the "all-reduce" glue) ----
    C = np.zeros((128, 128), dtype=np.float64)
    for i in range(N_CORES):
        C += r1.results[i]["p1"].astype(np.float64)
    # exact per-batch rowsums (feed the cancellation-heavy S1, which
    # cannot tolerate fp8; a 0.017 GFLOP reduction vs the 4.3 GFLOP
    # on-device Gram)
    Rall = u.astype(np.float64).sum(axis=1).T  # [128, B]
    W0d = W0.astype(np.float64)
    M = W0d @ W0d.T
    S2 = float(np.vdot(M, C))
    T = Rall.T @ W0d  # [B, 512]
    S1 = float(T.sum())
    s = S1 / np.sqrt(max(S2, 1e-12))
    sjh2 = (s / J) * T
    n2 = float((sjh2 * sjh2).sum())
    sj2 = (sjh2 / np.sqrt(max(n2, 1e-12))).reshape(B, J, D)
    # A[b][din, j] = sum_dd W0[din, j*16+dd] * sj2[b, j, dd];  fold s in
    A = np.einsum("dje,bje->bdj", W0d.reshape(DIN, J, D), sj2)
    As = (s * A).astype(bf16)  # [B, 128, 32]

    # ---- phase 2: logits/softmax/G ----
    nc2 = _get("p2")
    in2 = [
        {
            "u2": u2[i],
            "ut": utl[i],
            "A": np.ascontiguousarray(
                As[i * B_LOC : (i + 1) * B_LOC].transpose(1, 0, 2).reshape(DIN, -1)
            ),
        }
        for i in range(N_CORES)
    ]
    r2 = run_bass_kernel_spmd(
        nc2, in2, core_ids=list(range(N_CORES)), trace=PROFILE
    )
    if PROFILE:
        LAST_TIMES["phase2_ns"] = r2.exec_time_ns

    # ---- host: tiny fold + squash (O(B*J*D*DIN)) ----
    G = np.concatenate(
        [r2.results[i]["out"].astype(np.float64).reshape(B_LOC, J, DIN)
         for i in range(N_CORES)]
    )  # [B, J, 128]
    sjh3 = s * np.einsum("bjd,dje->bje", G, W0d.reshape(DIN, J, D))
    s2 = (sjh3 * sjh3).sum(axis=-1, keepdims=True) + 1e-7
    out = (np.sqrt(s2) / (1.0 + s2)) * sjh3
    return out.astype(np.float32)
